# revision 6
# baseline (speedup 1.0000x reference)
"""MDTA (Restormer transposed channel-attention) TRN2 Bass kernel.

Sharding: 8 cores = 4 batches x 2 row-halves (128 rows each, 1-row halo).

Per core: qkv 1x1 conv (PE, float32r) -> 3x3 depthwise conv (DVE fp16
scalar_tensor_tensor chains) -> PE transposes -> unnormalized per-head QK^T
partials + squared-norm partials -> tiny pair AllReduce (128KB) -> on-device
normalization + softmax -> attn @ v (PE fp16) -> 1x1 proj (PE fp16).

l2-normalize commutes with the pixel contraction:
  A[d,e] = (Q K^T)[d,e] / (|q_d| |k_e|)
so norms are applied to the [48,48] logits after the cross-core reduce.

Host permutes qkv channel order to [h0:q48|k48, h1:..., h2, h3, v:192] so
every on-device slice stays inside one <=128-partition tile.
"""
from contextlib import ExitStack

import numpy as np

import concourse.bacc as bacc
import concourse.bass as bass
import concourse.tile as tile
from concourse import mybir
from concourse.bass_utils import run_bass_kernel_spmd

dt = mybir.dt
F32, F32R, F16 = dt.float32, dt.float32r, dt.float16
MUL, ADD = mybir.AluOpType.mult, mybir.AluOpType.add
MAX = mybir.AluOpType.max
ACTF = mybir.ActivationFunctionType

B, C, H, W = 4, 192, 256, 256
NH, D = 4, 48
HR = H // 2            # 128 rows per core
WP = W + 2             # padded row width 258
R = 8                  # out rows per block
NBLK = HR // R         # 16
FIN = (R + 2) * WP     # 2580
FOUT = R * WP          # 2064
NPX = HR * W           # 32768
N_CORES = 8
NCH = 6                # qkv free chunks per block
CHW = FIN // NCH       # 430

# permuted-channel groups: 4x head(q48|k48) + v(128) + v(64)
GROUPS = [(0, 96), (96, 96), (192, 96), (288, 96), (384, 128), (512, 64)]

_CACHE = {}


def _emit(ctx, tc, y_d, x_d, wq_d, dw_d, tmpx_d, wp_d, id_d):
    nc = tc.nc
    wpool = ctx.enter_context(tc.tile_pool(name="weights", bufs=1))
    persist = ctx.enter_context(tc.tile_pool(name="persist", bufs=1))
    dram = ctx.enter_context(tc.tile_pool(name="dram", bufs=1, space="DRAM"))

    # weights
    wq0 = wpool.tile([128, 3 * C], F16)
    wq1 = wpool.tile([64, 3 * C], F16)
    nc.sync.dma_start(wq0[:], wq_d[0:128, :])
    nc.sync.dma_start(wq1[:], wq_d[128:192, :])
    dww = wpool.tile([128, 9 * 6], F32)
    for gi, (gs, gn) in enumerate(GROUPS):
        nc.sync.dma_start(dww[:gn, gi * 9:(gi + 1) * 9], dw_d[gs:gs + gn, :])
    wpj = wpool.tile([48, NH * 2 * C], F16)   # head h, o in [0,384): [48, 4*384]
    wpj32 = wpool.tile([48, NH * 2 * C], F32)
    nc.sync.dma_start(wpj32[:], wp_d[:])
    nc.vector.tensor_copy(wpj[:], wpj32[:])
    tmpx = wpool.tile([48, NH], F32)
    nc.sync.dma_start(tmpx[:], tmpx_d[:])
    id16 = wpool.tile([128, 128], F16)
    id32 = wpool.tile([128, 128], F32)
    nc.sync.dma_start(id32[:], id_d[:])
    nc.vector.tensor_copy(id16[:], id32[:])

    qk_acc = persist.tile([D, NH * D], F32)
    nrm_acc = persist.tile([96, 4 * NBLK], F32)
    nc.vector.memset(qk_acc[:], 0.0)
    nc.vector.memset(nrm_acc[:], 0.0)
    v_spill = dram.tile([C, HR, W], F16)
    at_f16 = persist.tile([D, NH * D], F16)

    # ---------------- phase 1 ----------------
    with ExitStack() as p1:
        xpool = p1.enter_context(tc.tile_pool(name="x", bufs=2))
        stage = p1.enter_context(tc.tile_pool(name="stage", bufs=2))
        cvout = p1.enter_context(tc.tile_pool(name="cvout", bufs=2))
        qktp = p1.enter_context(tc.tile_pool(name="qkt", bufs=2))
        scr = p1.enter_context(tc.tile_pool(name="scr", bufs=1))
        ps_mm = p1.enter_context(tc.tile_pool(name="psmm", bufs=2, space="PSUM"))
        ps_tr = p1.enter_context(tc.tile_pool(name="pstr", bufs=2, space="PSUM"))
        ps_qk = p1.enter_context(tc.tile_pool(name="psqk", bufs=1, space="PSUM"))

        for blk in range(NBLK):
            xt0 = xpool.tile([128, FIN], F16, tag="x0")
            xt1 = xpool.tile([64, FIN], F16, tag="x1")
            r0 = blk * R
            nc.sync.dma_start(xt0[:].rearrange("p (r w) -> p r w", w=WP), x_d[0:128, r0:r0 + R + 2, :])
            nc.sync.dma_start(xt1[:].rearrange("p (r w) -> p r w", w=WP), x_d[128:192, r0:r0 + R + 2, :])

            stg = []
            for gi, (gs, gn) in enumerate(GROUPS):
                st = stage.tile([128, FIN + 2], F16, tag=f"st{gi}")
                stg.append(st)
                for ch in range(NCH):
                    pt = ps_mm.tile([128, CHW], F32, tag="mm")
                    lo = ch * CHW
                    nc.tensor.matmul(
                        pt[:gn, :], wq0[:, gs:gs + gn],
                        xt0[:, lo:lo + CHW],
                        start=True, stop=False)
                    nc.tensor.matmul(
                        pt[:gn, :], wq1[:, gs:gs + gn],
                        xt1[:, lo:lo + CHW],
                        start=False, stop=True)
                    nc.scalar.copy(st[:gn, 1 + lo:1 + lo + CHW], pt[:gn, :])

            conv = []
            for gi, (gs, gn) in enumerate(GROUPS):
                st = stg[gi]
                co = cvout.tile([128, FOUT], F16, tag=f"co{gi}")
                conv.append(co)
                first = True
                for dy in (0, 1, 2):
                    for dx in (0, 1, 2):
                        tap = dy * 3 + dx
                        w_ap = dww[:gn, gi * 9 + tap:gi * 9 + tap + 1]
                        src = st[:gn, dy * WP + dx:dy * WP + dx + FOUT]
                        if first:
                            nc.vector.tensor_scalar_mul(co[:gn, :], src, w_ap)
                            first = False
                        else:
                            nc.vector.scalar_tensor_tensor(
                                co[:gn, :], src, w_ap, co[:gn, :], MUL, ADD)

            # v spill (interior cols)
            nc.sync.dma_start(
                v_spill[0:128, r0:r0 + R, :],
                conv[4][0:128, :].rearrange("p (r w) -> p r w", w=WP)[:, :, 1:1 + W])
            nc.sync.dma_start(
                v_spill[128:192, r0:r0 + R, :],
                conv[5][0:64, :].rearrange("p (r w) -> p r w", w=WP)[:, :, 1:1 + W])

            # squared-norm partials per head group
            for gi in range(4):
                sq = scr.tile([96, R * W], F16, tag="sq")
                nc.scalar.activation(
                    sq[:].rearrange("p (r w) -> p r w", w=W), conv[gi][0:96, :].rearrange("p (r w) -> p r w", w=WP)[:, :, 1:1 + W], ACTF.Square,
                    accum_out=nrm_acc[:, gi * NBLK + blk:gi * NBLK + blk + 1])

            # transposes + per-head QK^T
            qk_ps = [ps_qk.tile([D, D], F32, name=f"qk_ps{h}", tag=f"qk{h}") for h in range(NH)]
            nchunk = R * W // 128
            for gi in range(4):
                co = conv[gi]
                tt = qktp.tile([128, nchunk * 96], F16, tag=f"tt{gi}")
                for ck in range(nchunk):
                    row, half = divmod(ck, 2)
                    base = row * WP + 1 + half * 128
                    pt = ps_tr.tile([128, 96], F16, tag="tr")
                    nc.tensor.transpose(pt[:], co[:96, base:base + 128],
                                        id16[:96, :96])
                    nc.vector.tensor_copy(tt[:, ck * 96:(ck + 1) * 96], pt[:])
                for ck in range(nchunk):
                    nc.tensor.matmul(
                        qk_ps[gi][:],
                        tt[:, ck * 96:ck * 96 + D],
                        tt[:, ck * 96 + D:ck * 96 + 96],
                        start=(ck == 0), stop=(ck == nchunk - 1))
            for h in range(NH):
                nc.vector.tensor_add(
                    qk_acc[:, h * D:(h + 1) * D],
                    qk_acc[:, h * D:(h + 1) * D], qk_ps[h][:])

    # ---------------- allreduce ----------------
    nrm = persist.tile([96, 4], F32)
    for gi in range(4):
        nc.vector.tensor_reduce(
            nrm[:, gi:gi + 1], nrm_acc[:, gi * NBLK:(gi + 1) * NBLK],
            axis=mybir.AxisListType.X, op=ADD)
    cat = persist.tile([96, NH * D + 4], F32)
    nc.vector.memset(cat[:], 0.0)
    nc.vector.tensor_copy(cat[:D, 0:NH * D], qk_acc[:])
    nc.vector.tensor_copy(cat[:, NH * D:NH * D + 4], nrm[:])
    cc_in = dram.tile([96, NH * D + 4], F32)
    cc_out = dram.tile([96, NH * D + 4], F32)
    nc.sync.dma_start(cc_in[:], cat[:])
    nc.gpsimd.collective_compute(
        "AllReduce", ADD, replica_groups=[[0, 1], [2, 3], [4, 5], [6, 7]],
        ins=[cc_in.opt()], outs=[cc_out.opt()])
    red = persist.tile([96, NH * D + 4], F32)
    nc.sync.dma_start(red[:], cc_out[:])

    # ---------------- softmax ----------------
    with ExitStack() as p2:
        smp = p2.enter_context(tc.tile_pool(name="smp", bufs=2))
        ps_sm = p2.enter_context(tc.tile_pool(name="pssm", bufs=2, space="PSUM"))
        # recip norms per head group: rqr[96, 4]
        rt = persist.tile([96, 4], F32)
        nc.scalar.activation(rt[:], red[:, NH * D:NH * D + 4], ACTF.Sqrt)
        nc.vector.tensor_scalar_max(rt[:], rt[:], 1e-12)
        rqr = persist.tile([96, 4], F32)
        nc.vector.reciprocal(rqr[:], rt[:])
        for h in range(NH):
            # k-col recips to free dim: transpose [96,1] -> [1,96]
            ct_ps = ps_sm.tile([1, 96], F32, tag="ct")
            nc.tensor.transpose(ct_ps[:], rqr[:, h:h + 1],
                                id32[:96, :96])
            colv = smp.tile([1, 96], F16, tag="cv")
            nc.scalar.copy(colv[:], ct_ps[:])
            one48 = smp.tile([1, D], F16, tag="one")
            nc.vector.memset(one48[:], 1.0)
            bc_ps = ps_sm.tile([D, D], F32, tag="bc")
            nc.tensor.matmul(bc_ps[:], one48[:],
                             colv[:, D:96], start=True, stop=True)
            rowv = smp.tile([D, 1], F32, tag="rv")
            nc.vector.tensor_mul(rowv[:], rqr[:D, h:h + 1],
                                 tmpx[:, h:h + 1])
            logits = smp.tile([D, D], F32, tag="lg")
            nc.vector.scalar_tensor_tensor(
                logits[:], red[:D, h * D:(h + 1) * D], rowv[:], bc_ps[:],
                MUL, MUL)
            mx = smp.tile([D, 1], F32, tag="mx")
            nc.vector.tensor_reduce(mx[:], logits[:],
                                    axis=mybir.AxisListType.X, op=MAX)
            nmx = smp.tile([D, 1], F32, tag="nmx")
            nc.vector.tensor_scalar_mul(nmx[:], mx[:], -1.0)
            ex = smp.tile([D, D], F32, tag="ex")
            sm = smp.tile([D, 1], F32, tag="sm")
            nc.scalar.activation(ex[:], logits[:], ACTF.Exp, bias=nmx[:],
                                 scale=1.0, accum_out=sm[:])
            smr = smp.tile([D, 1], F32, tag="smr")
            nc.vector.reciprocal(smr[:], sm[:])
            a16 = smp.tile([D, D], F16, tag="a16")
            nc.vector.tensor_scalar_mul(a16[:], ex[:], smr[:])
            at_ps = ps_sm.tile([D, D], F16, tag="atp")
            nc.tensor.transpose(at_ps[:], a16[:], id16[:D, :D])
            nc.vector.tensor_copy(at_f16[:, h * D:(h + 1) * D], at_ps[:])

    # ---------------- phase 2: attn@v + proj ----------------
    with ExitStack() as p3:
        vp = p3.enter_context(tc.tile_pool(name="vp", bufs=3))
        op_ = p3.enter_context(tc.tile_pool(name="op", bufs=2))
        yp = p3.enter_context(tc.tile_pool(name="yp", bufs=2))
        ps_av = p3.enter_context(tc.tile_pool(name="psav", bufs=3, space="PSUM"))
        ps_pj = p3.enter_context(tc.tile_pool(name="pspj", bufs=2, space="PSUM"))
        NC2 = NPX // 512
        for ck in range(NC2):
            rr = ck * 2
            aos = []
            for h in range(NH):
                vt = vp.tile([D, 512], F16, tag=f"vt{h}")
                nc.sync.dma_start(vt[:].rearrange("p (r w) -> p r w", w=W), v_spill[h * D:(h + 1) * D, rr:rr + 2, :])
                av = ps_av.tile([D, 512], F32, tag="av")
                nc.tensor.matmul(av[:], at_f16[:, h * D:(h + 1) * D], vt[:],
                                 start=True, stop=True)
                ao = op_.tile([D, 512], F16, tag=f"ao{h}")
                nc.scalar.copy(ao[:], av[:])
                aos.append(ao)
            yt = yp.tile([128, 512], F32, tag="yt0")
            yt1 = yp.tile([64, 512], F32, tag="yt1")
            for mi, (ms, mn, ytile) in enumerate(((0, 128, yt), (128, 64, yt1))):
                pj = ps_pj.tile([128, 512], F32, tag="pj")
                for h in range(NH):
                    nc.tensor.matmul(
                        pj[:mn, :], wpj[:, h * 2 * C + ms:h * 2 * C + ms + mn],
                        aos[h][:], start=(h == 0), stop=(h == NH - 1))
                nc.scalar.copy(ytile[:mn, :], pj[:mn, :])
            nc.sync.dma_start(y_d[0:128, rr:rr + 2, :], yt[:].rearrange("p (r w) -> p r w", w=W))
            nc.sync.dma_start(y_d[128:192, rr:rr + 2, :], yt1[:].rearrange("p (r w) -> p r w", w=W))


def _build():
    if "nc" in _CACHE:
        return _CACHE["nc"]
    nc = bacc.Bacc("TRN2", target_bir_lowering=False, debug=False,
                   num_devices=N_CORES)
    x_d = nc.dram_tensor("x", [C, HR + 2, WP], F16, kind="ExternalInput").ap()
    wq_d = nc.dram_tensor("wqkvT", [C, 3 * C], F16, kind="ExternalInput").ap()
    dw_d = nc.dram_tensor("dww", [3 * C, 9], F32, kind="ExternalInput").ap()
    tmpx_d = nc.dram_tensor("tempx", [D, NH], F32, kind="ExternalInput").ap()
    wp_d = nc.dram_tensor("projT", [D, NH * 2 * C], F32, kind="ExternalInput").ap()
    id_d = nc.dram_tensor("ident", [128, 128], F32, kind="ExternalInput").ap()
    y_d = nc.dram_tensor("y", [C, HR, W], F32, kind="ExternalOutput").ap()
    with tile.TileContext(nc) as tc:
        with ExitStack() as ctx:
            _emit(ctx, tc, y_d, x_d, wq_d, dw_d, tmpx_d, wp_d, id_d)
    nc.compile()
    _CACHE["nc"] = nc
    return nc


def kernel(x, qkv_w, dw_w, temp, proj_w):
    x = np.asarray(x, np.float32)
    qkv_w = np.asarray(qkv_w, np.float32)
    dw_w = np.asarray(dw_w, np.float32)
    temp = np.asarray(temp, np.float32)
    proj_w = np.asarray(proj_w, np.float32)

    # channel permutation on the 576 qkv rows: [h: q48|k48]*4 + v192
    perm = []
    for h in range(NH):
        perm += list(range(h * D, (h + 1) * D))            # q head h
        perm += list(range(C + h * D, C + (h + 1) * D))    # k head h
    perm += list(range(2 * C, 3 * C))                      # v
    perm = np.array(perm)

    wqkvT = qkv_w[perm, :].T.copy()                        # [192, 576] permuted cols
    dww = dw_w[perm, 0].reshape(3 * C, 9).copy()           # [576, 9] permuted rows
    tempx = np.broadcast_to(temp.reshape(1, NH), (D, NH)).copy()  # [48, 4]
    # proj lhsT per head: rows = v-channels of head h (original v order
    # h*48..), cols = output channel slices [0:128],[128:192] concatenated
    wpjT = np.zeros((D, NH * 2 * C), np.float32)
    for h in range(NH):
        blockT = proj_w[:, h * D:(h + 1) * D].T            # [48, 192]
        wpjT[:, h * 2 * C:h * 2 * C + C] = blockT          # cols 0..191 (o 0:128 at 0:128, o 128:192 at 128:192)
    # device expects [48, h*384 + (0:128 -> o 0:128 | 128:192 -> o 128:192)]
    # blockT already [48, 192] with o contiguous; M-tiles slice [0:128],[128:192]
    ident = np.eye(128, dtype=np.float32)

    # pad x: [B, C, H, W] -> per-core [C, 130, 258] with zero halo+cols
    xp = np.zeros((B, C, H + 2, WP), np.float32)
    xp[:, :, 1:H + 1, 1:W + 1] = x
    in_maps = []
    for core in range(N_CORES):
        b, half = divmod(core, 2)
        r0 = half * HR
        in_maps.append({
            "x": np.ascontiguousarray(xp[b, :, r0:r0 + HR + 2, :]).astype(np.float16),
            "wqkvT": wqkvT.astype(np.float16), "dww": dww, "tempx": tempx,
            "projT": wpjT, "ident": ident,
        })

    nc = _build()
    res = run_bass_kernel_spmd(nc, in_maps, core_ids=list(range(N_CORES)))
    out = np.zeros((B, C, H, W), np.float32)
    for core in range(N_CORES):
        b, half = divmod(core, 2)
        out[b, :, half * HR:(half + 1) * HR, :] = res.results[core]["y"]
    return out


# revision 7
# speedup vs baseline: 1.5942x; 1.5942x over previous
"""MDTA (Restormer transposed channel-attention) TRN2 Bass kernel.

Sharding: 8 cores = 4 batches x 2 row-halves (128 rows each, 1-row halo).

Per core: qkv 1x1 conv (PE, float32r) -> 3x3 depthwise conv (DVE fp16
scalar_tensor_tensor chains) -> PE transposes -> unnormalized per-head QK^T
partials + squared-norm partials -> tiny pair AllReduce (128KB) -> on-device
normalization + softmax -> attn @ v (PE fp16) -> 1x1 proj (PE fp16).

l2-normalize commutes with the pixel contraction:
  A[d,e] = (Q K^T)[d,e] / (|q_d| |k_e|)
so norms are applied to the [48,48] logits after the cross-core reduce.

Host permutes qkv channel order to [h0:q48|k48, h1:..., h2, h3, v:192] so
every on-device slice stays inside one <=128-partition tile.
"""
from contextlib import ExitStack

import numpy as np

import concourse.bacc as bacc
import concourse.bass as bass
import concourse.tile as tile
from concourse import mybir
from concourse.bass_utils import run_bass_kernel_spmd

dt = mybir.dt
F32, F32R, F16 = dt.float32, dt.float32r, dt.float16
MUL, ADD = mybir.AluOpType.mult, mybir.AluOpType.add
MAX = mybir.AluOpType.max
ACTF = mybir.ActivationFunctionType

B, C, H, W = 4, 192, 256, 256
NH, D = 4, 48
HR = H // 2            # 128 rows per core
WP = W + 2             # padded row width 258
R = 8                  # out rows per block
NBLK = HR // R         # 16
FIN = (R + 2) * WP     # 2580
FOUT = R * WP          # 2064
NPX = HR * W           # 32768
N_CORES = 8
NCH = 6                # qkv free chunks per block
CHW = FIN // NCH       # 430

# permuted-channel groups: 4x head(q48|k48) + v(128) + v(64)
GROUPS = [(0, 96), (96, 96), (192, 96), (288, 96), (384, 128), (512, 64)]

_CACHE = {}


def _emit(ctx, tc, y_d, x_d, wq_d, dw_d, tmpx_d, wp_d, id_d):
    nc = tc.nc
    wpool = ctx.enter_context(tc.tile_pool(name="weights", bufs=1))
    persist = ctx.enter_context(tc.tile_pool(name="persist", bufs=1))
    dram = ctx.enter_context(tc.tile_pool(name="dram", bufs=1, space="DRAM"))

    # weights
    wq0 = wpool.tile([128, 3 * C], F16)
    wq1 = wpool.tile([64, 3 * C], F16)
    nc.sync.dma_start(wq0[:], wq_d[0:128, :])
    nc.sync.dma_start(wq1[:], wq_d[128:192, :])
    dww = wpool.tile([128, 9 * 6], F32)
    for gi, (gs, gn) in enumerate(GROUPS):
        nc.sync.dma_start(dww[:gn, gi * 9:(gi + 1) * 9], dw_d[gs:gs + gn, :])
    wpj = wpool.tile([48, NH * 2 * C], F16)   # head h, o in [0,384): [48, 4*384]
    wpj32 = wpool.tile([48, NH * 2 * C], F32)
    nc.sync.dma_start(wpj32[:], wp_d[:])
    nc.vector.tensor_copy(wpj[:], wpj32[:])
    tmpx = wpool.tile([48, NH], F32)
    nc.sync.dma_start(tmpx[:], tmpx_d[:])
    id16 = wpool.tile([128, 128], F16)
    id32 = wpool.tile([128, 128], F32)
    nc.sync.dma_start(id32[:], id_d[:])
    nc.vector.tensor_copy(id16[:], id32[:])

    qk_acc = persist.tile([D, NH * D], F32)
    nrm_acc = persist.tile([96, 4 * NBLK], F32)
    nc.vector.memset(qk_acc[:], 0.0)
    nc.vector.memset(nrm_acc[:], 0.0)
    v_spill = dram.tile([C, HR, W], F16)
    at_f16 = persist.tile([D, NH * D], F16)

    # ---------------- phase 1 ----------------
    with ExitStack() as p1:
        xpool = p1.enter_context(tc.tile_pool(name="x", bufs=2))
        stage = p1.enter_context(tc.tile_pool(name="stage", bufs=1))
        stage2 = p1.enter_context(tc.tile_pool(name="stage2", bufs=1))
        cvout = p1.enter_context(tc.tile_pool(name="cvout", bufs=2))
        qktp = p1.enter_context(tc.tile_pool(name="qkt", bufs=2))
        scr = p1.enter_context(tc.tile_pool(name="scr", bufs=1))
        ps_mm = p1.enter_context(tc.tile_pool(name="psmm", bufs=2, space="PSUM"))
        ps_tr = p1.enter_context(tc.tile_pool(name="pstr", bufs=2, space="PSUM"))
        ps_qk = p1.enter_context(tc.tile_pool(name="psqk", bufs=1, space="PSUM"))

        for blk in range(NBLK):
            xt0 = xpool.tile([128, FIN], F16, tag="x0")
            xt1 = xpool.tile([64, FIN], F16, tag="x1")
            r0 = blk * R
            nc.sync.dma_start(xt0[:].rearrange("p (r w) -> p r w", w=WP), x_d[0:128, r0:r0 + R + 2, :])
            nc.sync.dma_start(xt1[:].rearrange("p (r w) -> p r w", w=WP), x_d[128:192, r0:r0 + R + 2, :])

            stg = []
            stg2 = []
            for gi, (gs, gn) in enumerate(GROUPS):
                st = stage.tile([128, FIN + 2], F16, tag=f"st{gi}")
                st2 = stage2.tile([128, FIN], F16, name=f"st2_{gi}", tag=f"s2{gi}")
                stg.append(st)
                stg2.append(st2)
                for ch in range(NCH):
                    pt = ps_mm.tile([128, CHW], F32, tag="mm")
                    lo = ch * CHW
                    nc.tensor.matmul(
                        pt[:gn, :], wq0[:, gs:gs + gn],
                        xt0[:, lo:lo + CHW],
                        start=True, stop=False)
                    nc.tensor.matmul(
                        pt[:gn, :], wq1[:, gs:gs + gn],
                        xt1[:, lo:lo + CHW],
                        start=False, stop=True)
                    nc.scalar.copy(st[:gn, 1 + lo:1 + lo + CHW], pt[:gn, :])
                    nc.scalar.copy(st2[:gn, lo:lo + CHW], pt[:gn, :])

            conv = []
            for gi, (gs, gn) in enumerate(GROUPS):
                st = stg[gi]
                co = cvout.tile([128, FOUT], F16, tag=f"co{gi}")
                conv.append(co)
                first = True
                for dy in (0, 1, 2):
                    for dx in (0, 1, 2):
                        tap = dy * 3 + dx
                        w_ap = dww[:gn, gi * 9 + tap:gi * 9 + tap + 1]
                        if dx == 1:
                            src = stg2[gi][:gn, dy * WP:dy * WP + FOUT]
                        else:
                            src = st[:gn, dy * WP + dx:dy * WP + dx + FOUT]
                        if first:
                            nc.vector.tensor_scalar_mul(co[:gn, :], src, w_ap)
                            first = False
                        else:
                            nc.vector.scalar_tensor_tensor(
                                co[:gn, :], src, w_ap, co[:gn, :], MUL, ADD)

            # v spill (interior cols)
            nc.sync.dma_start(
                v_spill[0:128, r0:r0 + R, :],
                conv[4][0:128, :].rearrange("p (r w) -> p r w", w=WP)[:, :, 1:1 + W])
            nc.sync.dma_start(
                v_spill[128:192, r0:r0 + R, :],
                conv[5][0:64, :].rearrange("p (r w) -> p r w", w=WP)[:, :, 1:1 + W])

            # squared-norm partials per head group
            for gi in range(4):
                sq = scr.tile([96, R * W], F16, tag="sq")
                nc.scalar.activation(
                    sq[:].rearrange("p (r w) -> p r w", w=W), conv[gi][0:96, :].rearrange("p (r w) -> p r w", w=WP)[:, :, 1:1 + W], ACTF.Square,
                    accum_out=nrm_acc[:, gi * NBLK + blk:gi * NBLK + blk + 1])

            # transposes + per-head QK^T
            qk_ps = [ps_qk.tile([D, D], F32, name=f"qk_ps{h}", tag=f"qk{h}") for h in range(NH)]
            nchunk = R * W // 128
            for gi in range(4):
                co = conv[gi]
                tt = qktp.tile([128, nchunk * 96], F16, tag=f"tt{gi}")
                for ck in range(nchunk):
                    row, half = divmod(ck, 2)
                    base = row * WP + 1 + half * 128
                    pt = ps_tr.tile([128, 96], F16, tag="tr")
                    nc.tensor.transpose(pt[:], co[:96, base:base + 128],
                                        id16[:96, :96])
                    nc.vector.tensor_copy(tt[:, ck * 96:(ck + 1) * 96], pt[:])
                for ck in range(nchunk):
                    nc.tensor.matmul(
                        qk_ps[gi][:],
                        tt[:, ck * 96:ck * 96 + D],
                        tt[:, ck * 96 + D:ck * 96 + 96],
                        start=(ck == 0), stop=(ck == nchunk - 1))
            for h in range(NH):
                nc.vector.tensor_add(
                    qk_acc[:, h * D:(h + 1) * D],
                    qk_acc[:, h * D:(h + 1) * D], qk_ps[h][:])

    # ---------------- allreduce ----------------
    nrm = persist.tile([96, 4], F32)
    for gi in range(4):
        nc.vector.tensor_reduce(
            nrm[:, gi:gi + 1], nrm_acc[:, gi * NBLK:(gi + 1) * NBLK],
            axis=mybir.AxisListType.X, op=ADD)
    cat = persist.tile([96, NH * D + 4], F32)
    nc.vector.memset(cat[:], 0.0)
    nc.vector.tensor_copy(cat[:D, 0:NH * D], qk_acc[:])
    nc.vector.tensor_copy(cat[:, NH * D:NH * D + 4], nrm[:])
    cc_in = dram.tile([96, NH * D + 4], F32)
    cc_out = dram.tile([96, NH * D + 4], F32)
    nc.sync.dma_start(cc_in[:], cat[:])
    nc.gpsimd.collective_compute(
        "AllReduce", ADD, replica_groups=[[0, 1], [2, 3], [4, 5], [6, 7]],
        ins=[cc_in.opt()], outs=[cc_out.opt()])
    red = persist.tile([96, NH * D + 4], F32)
    nc.sync.dma_start(red[:], cc_out[:])

    # ---------------- softmax ----------------
    with ExitStack() as p2:
        smp = p2.enter_context(tc.tile_pool(name="smp", bufs=2))
        ps_sm = p2.enter_context(tc.tile_pool(name="pssm", bufs=2, space="PSUM"))
        # recip norms per head group: rqr[96, 4]
        rt = persist.tile([96, 4], F32)
        nc.scalar.activation(rt[:], red[:, NH * D:NH * D + 4], ACTF.Sqrt)
        nc.vector.tensor_scalar_max(rt[:], rt[:], 1e-12)
        rqr = persist.tile([96, 4], F32)
        nc.vector.reciprocal(rqr[:], rt[:])
        for h in range(NH):
            # k-col recips to free dim: transpose [96,1] -> [1,96]
            ct_ps = ps_sm.tile([1, 96], F32, tag="ct")
            nc.tensor.transpose(ct_ps[:], rqr[:, h:h + 1],
                                id32[:96, :96])
            colv = smp.tile([1, 96], F16, tag="cv")
            nc.scalar.copy(colv[:], ct_ps[:])
            one48 = smp.tile([1, D], F16, tag="one")
            nc.vector.memset(one48[:], 1.0)
            bc_ps = ps_sm.tile([D, D], F32, tag="bc")
            nc.tensor.matmul(bc_ps[:], one48[:],
                             colv[:, D:96], start=True, stop=True)
            rowv = smp.tile([D, 1], F32, tag="rv")
            nc.vector.tensor_mul(rowv[:], rqr[:D, h:h + 1],
                                 tmpx[:, h:h + 1])
            logits = smp.tile([D, D], F32, tag="lg")
            nc.vector.scalar_tensor_tensor(
                logits[:], red[:D, h * D:(h + 1) * D], rowv[:], bc_ps[:],
                MUL, MUL)
            mx = smp.tile([D, 1], F32, tag="mx")
            nc.vector.tensor_reduce(mx[:], logits[:],
                                    axis=mybir.AxisListType.X, op=MAX)
            nmx = smp.tile([D, 1], F32, tag="nmx")
            nc.vector.tensor_scalar_mul(nmx[:], mx[:], -1.0)
            ex = smp.tile([D, D], F32, tag="ex")
            sm = smp.tile([D, 1], F32, tag="sm")
            nc.scalar.activation(ex[:], logits[:], ACTF.Exp, bias=nmx[:],
                                 scale=1.0, accum_out=sm[:])
            smr = smp.tile([D, 1], F32, tag="smr")
            nc.vector.reciprocal(smr[:], sm[:])
            a16 = smp.tile([D, D], F16, tag="a16")
            nc.vector.tensor_scalar_mul(a16[:], ex[:], smr[:])
            at_ps = ps_sm.tile([D, D], F16, tag="atp")
            nc.tensor.transpose(at_ps[:], a16[:], id16[:D, :D])
            nc.vector.tensor_copy(at_f16[:, h * D:(h + 1) * D], at_ps[:])

    # ---------------- phase 2: attn@v + proj ----------------
    with ExitStack() as p3:
        vp = p3.enter_context(tc.tile_pool(name="vp", bufs=3))
        op_ = p3.enter_context(tc.tile_pool(name="op", bufs=2))
        yp = p3.enter_context(tc.tile_pool(name="yp", bufs=2))
        ps_av = p3.enter_context(tc.tile_pool(name="psav", bufs=3, space="PSUM"))
        ps_pj = p3.enter_context(tc.tile_pool(name="pspj", bufs=2, space="PSUM"))
        NC2 = NPX // 512
        for ck in range(NC2):
            rr = ck * 2
            aos = []
            for h in range(NH):
                vt = vp.tile([D, 512], F16, tag=f"vt{h}")
                nc.sync.dma_start(vt[:].rearrange("p (r w) -> p r w", w=W), v_spill[h * D:(h + 1) * D, rr:rr + 2, :])
                av = ps_av.tile([D, 512], F32, tag="av")
                nc.tensor.matmul(av[:], at_f16[:, h * D:(h + 1) * D], vt[:],
                                 start=True, stop=True)
                ao = op_.tile([D, 512], F16, tag=f"ao{h}")
                nc.scalar.copy(ao[:], av[:])
                aos.append(ao)
            yt = yp.tile([128, 512], F32, tag="yt0")
            yt1 = yp.tile([64, 512], F32, tag="yt1")
            for mi, (ms, mn, ytile) in enumerate(((0, 128, yt), (128, 64, yt1))):
                pj = ps_pj.tile([128, 512], F32, tag="pj")
                for h in range(NH):
                    nc.tensor.matmul(
                        pj[:mn, :], wpj[:, h * 2 * C + ms:h * 2 * C + ms + mn],
                        aos[h][:], start=(h == 0), stop=(h == NH - 1))
                nc.scalar.copy(ytile[:mn, :], pj[:mn, :])
            nc.sync.dma_start(y_d[0:128, rr:rr + 2, :], yt[:].rearrange("p (r w) -> p r w", w=W))
            nc.sync.dma_start(y_d[128:192, rr:rr + 2, :], yt1[:].rearrange("p (r w) -> p r w", w=W))


def _build():
    if "nc" in _CACHE:
        return _CACHE["nc"]
    nc = bacc.Bacc("TRN2", target_bir_lowering=False, debug=False,
                   num_devices=N_CORES)
    x_d = nc.dram_tensor("x", [C, HR + 2, WP], F16, kind="ExternalInput").ap()
    wq_d = nc.dram_tensor("wqkvT", [C, 3 * C], F16, kind="ExternalInput").ap()
    dw_d = nc.dram_tensor("dww", [3 * C, 9], F32, kind="ExternalInput").ap()
    tmpx_d = nc.dram_tensor("tempx", [D, NH], F32, kind="ExternalInput").ap()
    wp_d = nc.dram_tensor("projT", [D, NH * 2 * C], F32, kind="ExternalInput").ap()
    id_d = nc.dram_tensor("ident", [128, 128], F32, kind="ExternalInput").ap()
    y_d = nc.dram_tensor("y", [C, HR, W], F32, kind="ExternalOutput").ap()
    with tile.TileContext(nc) as tc:
        with ExitStack() as ctx:
            _emit(ctx, tc, y_d, x_d, wq_d, dw_d, tmpx_d, wp_d, id_d)
    nc.compile()
    _CACHE["nc"] = nc
    return nc


def kernel(x, qkv_w, dw_w, temp, proj_w):
    x = np.asarray(x, np.float32)
    qkv_w = np.asarray(qkv_w, np.float32)
    dw_w = np.asarray(dw_w, np.float32)
    temp = np.asarray(temp, np.float32)
    proj_w = np.asarray(proj_w, np.float32)

    # channel permutation on the 576 qkv rows: [h: q48|k48]*4 + v192
    perm = []
    for h in range(NH):
        perm += list(range(h * D, (h + 1) * D))            # q head h
        perm += list(range(C + h * D, C + (h + 1) * D))    # k head h
    perm += list(range(2 * C, 3 * C))                      # v
    perm = np.array(perm)

    wqkvT = qkv_w[perm, :].T.copy()                        # [192, 576] permuted cols
    dww = dw_w[perm, 0].reshape(3 * C, 9).copy()           # [576, 9] permuted rows
    tempx = np.broadcast_to(temp.reshape(1, NH), (D, NH)).copy()  # [48, 4]
    # proj lhsT per head: rows = v-channels of head h (original v order
    # h*48..), cols = output channel slices [0:128],[128:192] concatenated
    wpjT = np.zeros((D, NH * 2 * C), np.float32)
    for h in range(NH):
        blockT = proj_w[:, h * D:(h + 1) * D].T            # [48, 192]
        wpjT[:, h * 2 * C:h * 2 * C + C] = blockT          # cols 0..191 (o 0:128 at 0:128, o 128:192 at 128:192)
    # device expects [48, h*384 + (0:128 -> o 0:128 | 128:192 -> o 128:192)]
    # blockT already [48, 192] with o contiguous; M-tiles slice [0:128],[128:192]
    ident = np.eye(128, dtype=np.float32)

    # pad x: [B, C, H, W] -> per-core [C, 130, 258] with zero halo+cols
    xp = np.zeros((B, C, H + 2, WP), np.float32)
    xp[:, :, 1:H + 1, 1:W + 1] = x
    in_maps = []
    for core in range(N_CORES):
        b, half = divmod(core, 2)
        r0 = half * HR
        in_maps.append({
            "x": np.ascontiguousarray(xp[b, :, r0:r0 + HR + 2, :]).astype(np.float16),
            "wqkvT": wqkvT.astype(np.float16), "dww": dww, "tempx": tempx,
            "projT": wpjT, "ident": ident,
        })

    nc = _build()
    res = run_bass_kernel_spmd(nc, in_maps, core_ids=list(range(N_CORES)))
    out = np.zeros((B, C, H, W), np.float32)
    for core in range(N_CORES):
        b, half = divmod(core, 2)
        out[b, :, half * HR:(half + 1) * HR, :] = res.results[core]["y"]
    return out
